# revision 14
# baseline (speedup 1.0000x reference)
"""Bag-of-words classifier kernel for Trainium2 (8 NeuronCores, data-parallel).

Math: logits[b, c] = sum_s [ids[b,s] != 0] * W[c, ids[b,s]] + b[c].

Per core (64 rows, 32768 tokens), the gather runs on the Ant dma_gather
SWDGE instruction with a radix-2 table:
  - table rows (256B stride) hold vocab pairs {2q, 2q+1} as 4 f32:
    [W0[2q], W1[2q], W0[2q+1], W1[2q+1]]; row 0 slots 0:2 zeroed (pad mask).
  - gather index = id >> 1 (max 25128, fits int16), precomputed on host in
    the ucode's wrapped idx layout, elem_size = 4 f32 (16B).
  - DVE selects the pair by lo = id & 1 (host-precomputed selector mask),
    reduces over tokens, folds partition halves, adds bias.

Eight gathers (4x6144 then 4x2048 idxs) ride the four SWDGE queues
(queue q is served by Q7 cpu pair 2q/2q+1), so all eight Q7 cpus
generate descriptors concurrently: two back-to-back waves of four
gathers, ~65us total; the small second wave leaves little data in
flight at the end, trimming the DMA drain tail.
dynamic_dma_scratch_size=98304 sizes the descriptor rings so all four
queues' gathers coexist with depth to spare (32KB serializes them to
two at a time; 96KB measures ~1us faster than 64KB).

Token order i <-> (p = i % 128, j = i // 128 within chunk); partition
2r holds row r tokens [0,256), partition 2r+1 tokens [256,512), so the
partition-half fold is a quadrant-local DVE stream_shuffle (no DMA hop).
Host reads rows from even partitions of the [128, C] output.
"""

import os

import numpy as np

import concourse.bass as bass
import concourse.tile as tile
from concourse import bacc, mybir
from concourse.bass_utils import run_bass_kernel_spmd

B, S, V, C = 512, 512, 50257, 2
NCORES = 8
RPC = B // NCORES  # rows per core = 64
P = 128
NI = RPC * S  # gathers per core = 32768
QROWS = (V + 1) // 2 + 1  # 25130 radix-2 table rows (incl. pad row for odd V)

_sizes_env = os.environ.get(
    "BOW_SIZES", "6144,6144,6144,6144,2048,2048,2048,2048"
)
SIZES = [int(x) for x in _sizes_env.split(",")]
assert sum(SIZES) == NI and all(sz % 128 == 0 for sz in SIZES)
CHUNKS = len(SIZES)
NQUEUES = int(os.environ.get("BOW_NQUEUES", "4"))
SCRATCH = int(os.environ.get("BOW_SCRATCH", "98304"))

LAST_EXEC_TIME_NS = None

_cache = {}


def _raw_dma_gather(
    nc, out_ap, in_ap, idxs_ap, num_idxs, elem_size, elem_step, queue_num=0
):
    """bass dma_gather without the elem_size*dtype%256 assert (ucode handles
    small elem_size with a 256B-multiple row stride; HW-verified)."""
    from concourse.bass import exact_div

    g = nc.gpsimd
    assert idxs_ap.dtype == mybir.dt.int16
    stride_bytes = elem_step * mybir.dt.size(in_ap.dtype)
    stride_bytes_256 = exact_div(stride_bytes, 256)
    _in_ap = g.lower_ap_dma(in_ap, for_custom_bir_dma=True)
    _idxs_ap = g.lower_ap(idxs_ap)
    _out_ap = g.lower_ap(out_ap)
    return g.add_instruction(
        mybir.InstDMAGatherAnt(
            name=nc.get_next_instruction_name(),
            ins=[*_in_ap, _idxs_ap, g.lower_val_access(g.to_reg(num_idxs))],
            outs=[_out_ap],
            transpose=False,
            num_idxs=num_idxs,
            elem_size=elem_size,
            stride_bytes_256=stride_bytes_256,
            gen_mode=0,
            single_packet=False,
            queue_num=queue_num,
            sbuf_tokens_per_rank=0,
            sbuf_free_dim_per_rank=0,
            sbuf_free_dim_pad_per_rank=0,
            sbuf_byte_offset=0,
        )
    )


def _build(chunks: int) -> bass.Bass:
    nc = bacc.Bacc(
        "TRN2",
        target_bir_lowering=False,
        debug=False,
        num_devices=NCORES,
        num_swdge_queues=NQUEUES,
        dynamic_dma_scratch_size=SCRATCH,
    )
    # host-precomputed wrapped int16 idxs (id >> 1), one tensor per chunk;
    # only the 32-partition window of the chunk's queue cpu pair is loaded
    idx_d = [
        nc.dram_tensor(f"idx{k}", [32, SIZES[k] // 16], mybir.dt.int16,
                       kind="ExternalInput")
        for k in range(chunks)
    ]
    # host-precomputed lo-bit selector, replicated per class: [P, NI//P, C]
    lo_d = nc.dram_tensor("lo2", [P, NI // P, C], mybir.dt.float16,
                          kind="ExternalInput")
    tab16 = bool(int(os.environ.get("BOW_TAB16", "0")))
    if tab16:
        tab_d = nc.dram_tensor("table", [QROWS, 128], mybir.dt.float16,
                               kind="ExternalInput")
    else:
        tab_d = nc.dram_tensor("table", [QROWS, 64], mybir.dt.float32,
                               kind="ExternalInput")
    bias_d = nc.dram_tensor("bias", [P, C], mybir.dt.float32,
                            kind="ExternalInput")
    out_d = nc.dram_tensor("out", [P, C], mybir.dt.float32,
                           kind="ExternalOutput")

    with tile.TileContext(nc) as tc:
        with tc.tile_pool(name="sbuf", bufs=1) as pool:
            lo2 = pool.tile([P, NI // P, C], mybir.dt.float16)
            r = pool.tile([P, C], mybir.dt.float32)
            rsh = pool.tile([P, C], mybir.dt.float32)
            bias_sb = pool.tile([P, C], mybir.dt.float32)
            out_sb = pool.tile([P, C], mybir.dt.float32)

            idx16_k = [
                pool.tile([P, SIZES[k] // 16], mybir.dt.int16, name=f"idx16_{k}")
                for k in range(chunks)
            ]
            gdt = mybir.dt.float16 if tab16 else mybir.dt.float32
            g_k = [
                pool.tile([P, SIZES[k] // P, 4], gdt, name=f"g{k}")
                for k in range(chunks)
            ]
            vals_k = [
                pool.tile([P, SIZES[k] // P, C], mybir.dt.float32, name=f"vals{k}")
                for k in range(chunks)
            ]
            rk = [
                pool.tile([P, C], mybir.dt.float32, name=f"rk{k}")
                for k in range(chunks)
            ]

            for k in range(chunks):
                q = (k + 1) % NQUEUES
                nc.sync.dma_start(out=idx16_k[k][32 * q : 32 * q + 32, :],
                                  in_=idx_d[k][:])
            nc.sync.dma_start(out=lo2[:], in_=lo_d[:])
            nc.sync.dma_start(out=bias_sb[:], in_=bias_d[:])

            warm_g = pool.tile([P, 1, 4], mybir.dt.float32, name="warm_g")
            use_warm = bool(int(os.environ.get("BOW_WARM", "0")))
            if use_warm:
                # chunk NQUEUES-1 is the one whose idx window (partitions
                # 0-31) queue 0's cpu pair reads, so its idxs are valid here
                kw = NQUEUES - 1
                _raw_dma_gather(
                    nc, warm_g[:], tab_d[:, 0:4],
                    idx16_k[kw][:, 0:8], 128, 4, 64, queue_num=0,
                )
            # queue (k+1)%N: both chunks dispatch immediately and run
            # concurrently on distinct SWDGE queues
            for k in range(chunks):
                _raw_dma_gather(
                    nc,
                    g_k[k][:],
                    tab_d[:, 0:4],
                    idx16_k[k][:],
                    SIZES[k],
                    4,
                    128 if tab16 else 64,
                    queue_num=(k + 1) % NQUEUES,
                )

            joff = [0]
            for sz in SIZES:
                joff.append(joff[-1] + sz // P)
            for k in range(chunks):
                js = slice(joff[k], joff[k + 1])
                # vals = g02 + lo2 * (g24 - g02)
                nc.vector.tensor_tensor(
                    out=vals_k[k][:],
                    in0=g_k[k][:, :, 2:4],
                    in1=g_k[k][:, :, 0:2],
                    op=mybir.AluOpType.subtract,
                )
                nc.vector.tensor_tensor(
                    out=vals_k[k][:],
                    in0=vals_k[k][:],
                    in1=lo2[:, js, :],
                    op=mybir.AluOpType.mult,
                )
                nc.vector.tensor_tensor(
                    out=vals_k[k][:],
                    in0=vals_k[k][:],
                    in1=g_k[k][:, :, 0:2],
                    op=mybir.AluOpType.add,
                )
                # rk[p, c] = sum_j vals[p, j, c]
                nc.vector.tensor_reduce(
                    out=rk[k][:],
                    in_=vals_k[k][:].transpose([0, 2, 1]),
                    axis=mybir.AxisListType.X,
                    op=mybir.AluOpType.add,
                )
            if chunks == 1:
                r = rk[0]
            else:
                level = list(rk)
                tmp_id = 0
                while len(level) > 1:
                    nxt = []
                    for i in range(0, len(level) - 1, 2):
                        dst = (
                            r
                            if len(level) == 2
                            else pool.tile([P, C], mybir.dt.float32,
                                           name=f"rt{tmp_id}")
                        )
                        tmp_id += 1
                        nc.vector.tensor_tensor(
                            out=dst[:], in0=level[i][:], in1=level[i + 1][:],
                            op=mybir.AluOpType.add,
                        )
                        nxt.append(dst)
                    if len(level) % 2 == 1:
                        nxt.append(level[-1])
                    level = nxt
            # fold partition-half pairs (2r, 2r+1) + bias, all on DVE
            nc.vector.stream_shuffle(rsh[:], r[:], [i ^ 1 for i in range(32)])
            nc.vector.tensor_tensor(
                out=out_sb[:], in0=r[:], in1=rsh[:], op=mybir.AluOpType.add
            )
            nc.vector.tensor_tensor(
                out=out_sb[:], in0=out_sb[:], in1=bias_sb[:], op=mybir.AluOpType.add
            )
            nc.sync.dma_start(out=out_d[:], in_=out_sb[:])
    nc.compile()
    return nc


def _host_layouts(ids_shard: np.ndarray, chunks: int):
    """ids_shard [RPC, S] int32 -> (lo2 [128, 256, C] f32,
    idx16 chunk list [128, NIC//16] i16)."""
    ids_nat = ids_shard.reshape(P, NI // P)  # partition 2r+h = row r half h
    lo2 = np.broadcast_to(
        (ids_nat & 1).astype(np.float16)[:, :, None], (P, NI // P, C)
    ).copy()
    idxw = (ids_nat >> 1).astype(np.int16)  # [128, 256] in natural layout
    out = []
    col = 0
    for k in range(chunks):
        jc = SIZES[k] // P
        sub = idxw[:, col : col + jc]  # [128, jc]
        col += jc
        a = sub.reshape(8, 16, jc)  # (p//16, p%16, jj)
        t = a.transpose(1, 2, 0).reshape(16, jc * 8)  # [16, NIC//16]
        out.append(np.ascontiguousarray(np.tile(t, (2, 1))))  # cpu pair window
    return np.ascontiguousarray(lo2), out


def _build_table(W: np.ndarray) -> np.ndarray:
    tab16 = bool(int(os.environ.get("BOW_TAB16", "0")))
    dt = np.float16 if tab16 else np.float32
    Wt = np.zeros((2 * QROWS, 2), dtype=dt)
    Wt[:V] = W.astype(dt).T
    Wt[0] = 0.0  # pad token contributes nothing
    cols = 128 if tab16 else 64
    table = np.zeros((QROWS, cols), dtype=dt)
    table[:, 0:4] = Wt.reshape(QROWS, 4)
    return table


def kernel(input_ids: np.ndarray, W: np.ndarray, b: np.ndarray) -> np.ndarray:
    global LAST_EXEC_TIME_NS
    ids = np.ascontiguousarray(np.asarray(input_ids, dtype=np.int32))
    table = _build_table(np.asarray(W, dtype=np.float32))
    bias = np.ascontiguousarray(
        np.tile(np.asarray(b, dtype=np.float32)[None, :], (P, 1))
    )

    if "nc" not in _cache:
        _cache["nc"] = _build(CHUNKS)
    nc = _cache["nc"]

    in_maps = []
    for c in range(NCORES):
        lo2, idx_chunks = _host_layouts(ids[c * RPC : (c + 1) * RPC], CHUNKS)
        m = {"lo2": lo2, "table": table, "bias": bias}
        for k in range(CHUNKS):
            m[f"idx{k}"] = idx_chunks[k]
        in_maps.append(m)

    trace = bool(int(os.environ.get("BOW_TRACE", "0")))
    res = run_bass_kernel_spmd(nc, in_maps, list(range(NCORES)), trace=trace)
    LAST_EXEC_TIME_NS = res.exec_time_ns

    out = np.concatenate(
        [res.results[i]["out"][0::2] for i in range(NCORES)], axis=0
    )
    return np.ascontiguousarray(out.astype(np.float32))
